# revision 3
# baseline (speedup 1.0000x reference)
"""Tensor-parallel self-attention (full-attention + rope) Bass kernel, TRN2.

Sharding: 16 heads over 8 cores (2 heads/core). Each core computes its heads'
QKV projections, rope, full attention, and its partial output projection
(rows of Wo for its heads); the host sums the 8 partial outputs (the
all-reduce of the tensor-parallel pattern, done at gather time).

v2 (bf16): all matmul operands are bf16 (same PE rate as f32r but enables
fast-weight-load, halves DMA/SBUF), PSUM accumulation fp32.

Per-core layouts (tokens on the free axis):
  xT      [D=2048, B*T=4096] bf16   x transposed (host-prepped), replicated
  wq/wk   [2048, 256] bf16          head-column shard; within each head the
                                    128 cols are permuted so rope pairs sit
                                    16-even+16-odd per 32-partition quadrant
  wv      [2048, 256] bf16          natural column shard
  wo      [256, 2048] bf16          natural row shard
  csa     [128, 2048] bf16          cos rows replicated on both quadrant halves
  csb     [128, 2048] bf16          [-sin ; +sin] per quadrant half
  ones    [128, 128] bf16           all-ones (softmax denominator matmul)

Rope per tile (no cross-partition DMA): with the quadrant pairing,
  out = pps*csa - stream_shuffle(pps*csb, swap 16<->16 within quadrants)
gives both rotated halves in one subtract (sign folded into csb).

Pipeline per batch: A) QKV projections (512-token blocks, x prefetched on the
ACT dma queue); B) per (head, 512-query block): s=kT.T@q -> exp (ACT, bf16
out) -> o += v.T@e, denom += ones.T@e (software-pipelined 2 deep);
oT = o * recip(denom); C) per 128-token tile: y = sum_h oT_h.T@wo_h,
DVE-copied to a [128,2048] bf16 staging tile, one row-contiguous DMA out.
"""

import sys

sys.path.insert(0, "/opt/trn_rl_repo")

import numpy as np
import ml_dtypes

import concourse.bass as bass
import concourse.mybir as mybir
import concourse.tile as tile
from concourse import bacc
from concourse.bass_utils import run_bass_kernel_spmd

B, T, D = 2, 2048, 2048
NH, HD = 16, 128
NCORES = 8
HPC = NH // NCORES          # heads per core = 2
CPC = HPC * HD              # proj columns per core = 256
BT = B * T                  # 4096 tokens
P = 128
TBLK = 512                  # phase-A token block
NBLK = T // TBLK            # 4 blocks per batch
DC = D // P                 # 16 contraction chunks
IBLK = 512                  # phase-B query block
NIB = T // IBLK             # 4 i-blocks per batch
NJT = T // P                # 16 key tiles per batch
SCALE = 1.0 / float(np.sqrt(HD))
SWAP_MASK = list(range(16, 32)) + list(range(16))

f32 = mybir.dt.float32
bf16 = mybir.dt.bfloat16
npbf = ml_dtypes.bfloat16

_compiled = {}

# exposed for test.py
last_results = None


def _build():
    nc = bacc.Bacc("TRN2", target_bir_lowering=False, debug=False)

    xT_d = nc.dram_tensor("xT", [D, BT], bf16, kind="ExternalInput").ap()
    wq_d = nc.dram_tensor("wq", [D, CPC], bf16, kind="ExternalInput").ap()
    wk_d = nc.dram_tensor("wk", [D, CPC], bf16, kind="ExternalInput").ap()
    wv_d = nc.dram_tensor("wv", [D, CPC], bf16, kind="ExternalInput").ap()
    wo_d = nc.dram_tensor("wo", [CPC, D], bf16, kind="ExternalInput").ap()
    csa_d = nc.dram_tensor("csa", [P, T], bf16, kind="ExternalInput").ap()
    csb_d = nc.dram_tensor("csb", [P, T], bf16, kind="ExternalInput").ap()
    ones_d = nc.dram_tensor("ones", [P, P], bf16, kind="ExternalInput").ap()
    y_d = nc.dram_tensor("y", [BT, D], bf16, kind="ExternalOutput").ap()

    with tile.TileContext(nc) as tc:
        _emit(nc, tc, xT_d, wq_d, wk_d, wv_d, wo_d, csa_d, csb_d, ones_d, y_d)
    nc.compile()
    return nc


def _emit(nc, tc, xT_d, wq_d, wk_d, wv_d, wo_d, csa_d, csb_d, ones_d, y_d):
    from contextlib import ExitStack

    Exp = mybir.ActivationFunctionType.Exp
    mult = mybir.AluOpType.mult
    sub = mybir.AluOpType.subtract

    with ExitStack() as ctx:
        const = ctx.enter_context(tc.tile_pool(name="const", bufs=1))
        state = ctx.enter_context(tc.tile_pool(name="state", bufs=1))

        wq_sb = const.tile([P, DC * CPC], bf16, tag="wq")
        wk_sb = const.tile([P, DC * CPC], bf16, tag="wk")
        wv_sb = const.tile([P, DC * CPC], bf16, tag="wv")
        wo_sb = const.tile([P, HPC * D], bf16, tag="wo")
        csa_sb = const.tile([P, T], bf16, tag="csa")
        csb_sb = const.tile([P, T], bf16, tag="csb")
        ones_sb = const.tile([P, P], bf16, tag="ones")

        # weights split per contraction chunk, issued in consumption order so
        # the first matmul starts ~2us in instead of waiting for monolithic
        # transfers
        for dc in range(DC):
            r = slice(dc * P, (dc + 1) * P)
            c = slice(dc * CPC, (dc + 1) * CPC)
            nc.sync.dma_start(wq_sb[:, c], wq_d[r, :])
            nc.sync.dma_start(wk_sb[:, c], wk_d[r, :])
            if dc == 0:
                nc.sync.dma_start(csa_sb[:], csa_d[:])
                nc.sync.dma_start(csb_sb[:], csb_d[:])
        for dc in range(DC):
            nc.sync.dma_start(
                wv_sb[:, dc * CPC:(dc + 1) * CPC], wv_d[dc * P:(dc + 1) * P, :])
        nc.sync.dma_start(ones_sb[:], ones_d[:])
        for h in range(HPC):
            for hf in range(2):
                c = slice(h * D + hf * (D // 2), h * D + (hf + 1) * (D // 2))
                nc.sync.dma_start(
                    wo_sb[:, c],
                    wo_d[h * P:(h + 1) * P, hf * (D // 2):(hf + 1) * (D // 2)])

        qT_sb = state.tile([P, HPC * T], bf16, tag="qT")
        kT_sb = state.tile([P, HPC * T], bf16, tag="kT")
        v_sb = state.tile([P, NJT * CPC], bf16, tag="v")
        oT_sb = state.tile([P, HPC * T], bf16, tag="oT")

        xpool = ctx.enter_context(tc.tile_pool(name="xa", bufs=3))
        xT_src = xT_d.rearrange("(dc p) t -> p dc t", p=P)
        ablocks = [(b, blk) for b in range(B) for blk in range(NBLK)]
        xtiles = {}

        def issue_x(i):
            # x DMAs ride the ACT hwdge queue (idle during phase A, and in
            # phase B the next batch's prefetch lands between exp calls)
            if i >= len(ablocks):
                return
            bb, blk = ablocks[i]
            xt = xpool.tile([P, DC * TBLK], bf16, tag="x")
            xr = xt[:].rearrange("p (dc t) -> p dc t", dc=DC)
            t0 = bb * T + blk * TBLK
            for c0 in range(0, DC, 4):
                nc.scalar.dma_start(
                    xr[:, c0:c0 + 4, :], xT_src[:, c0:c0 + 4, t0:t0 + TBLK])
            xtiles[i] = xt

        issue_x(0)
        issue_x(1)

        wqr = wq_sb[:].rearrange("p (dc c) -> p dc c", dc=DC)
        wkr = wk_sb[:].rearrange("p (dc c) -> p dc c", dc=DC)
        wvr = wv_sb[:].rearrange("p (dc c) -> p dc c", dc=DC)

        for b in range(B):
            g0 = b * T

            with tc.tile_pool(name=f"ra{b}", bufs=6) as rpool, \
                 tc.tile_pool(name=f"qk_ps{b}", bufs=3, space="PSUM") as qkps, \
                 tc.tile_pool(name=f"v_ps{b}", bufs=2, space="PSUM") as vps:
                for blk in range(NBLK):
                    idx = b * NBLK + blk
                    issue_x(idx + 2)
                    xt = xtiles.pop(idx)
                    xr = xt[:].rearrange("p (dc t) -> p dc t", dc=DC)
                    t0 = blk * TBLK
                    for h in range(HPC):
                        for wr, dst in ((wqr, qT_sb), (wkr, kT_sb)):
                            pps = qkps.tile([P, TBLK], f32, tag="qk")
                            for dc in range(DC):
                                nc.tensor.matmul(
                                    pps[:],
                                    wr[:, dc, h * HD:(h + 1) * HD],
                                    xr[:, dc, :],
                                    start=(dc == 0), stop=(dc == DC - 1))
                            ra = rpool.tile([P, TBLK], f32, tag="ra")
                            rb = rpool.tile([P, TBLK], f32, tag="rb")
                            rs = rpool.tile([P, TBLK], f32, tag="rs")
                            nc.vector.tensor_tensor(
                                ra[:], pps[:], csa_sb[:, t0:t0 + TBLK], mult)
                            nc.vector.tensor_tensor(
                                rb[:], pps[:], csb_sb[:, t0:t0 + TBLK], mult)
                            nc.vector.stream_shuffle(rs[:], rb[:], SWAP_MASK)
                            nc.vector.tensor_tensor(
                                dst[:, h * T + t0:h * T + t0 + TBLK],
                                ra[:], rs[:], sub)
                    for vi in range(2):
                        vp = vps.tile([P, 2 * CPC], f32, tag="v")
                        for tl2 in range(2):
                            tl = vi * 2 + tl2
                            for dc in range(DC):
                                nc.tensor.matmul(
                                    vp[:, tl2 * CPC:(tl2 + 1) * CPC],
                                    xr[:, dc, tl * P:(tl + 1) * P],
                                    wvr[:, dc, :],
                                    start=(dc == 0), stop=(dc == DC - 1))
                        j0 = blk * 4 + vi * 2
                        nc.vector.tensor_copy(
                            v_sb[:, j0 * CPC:(j0 + 2) * CPC], vp[:])

            with tc.tile_pool(name=f"e{b}", bufs=5) as epool, \
                 tc.tile_pool(name=f"rc{b}", bufs=4) as rcpool, \
                 tc.tile_pool(name=f"yb{b}", bufs=3) as ypool, \
                 tc.tile_pool(name=f"s_ps{b}", bufs=2, space="PSUM") as bps, \
                 tc.tile_pool(name=f"od_ps{b}", bufs=2, space="PSUM") as odn:
                for ib in range(NIB):
                    i0 = ib * IBLK
                    for h in range(HPC):
                        q_sl = qT_sb[:, h * T + i0:h * T + i0 + IBLK]
                        op = odn.tile([P, IBLK], f32, tag="o")
                        dn = odn.tile([P, IBLK], f32, tag="d")
                        es = {}
                        # software-pipeline 2 deep: s(jt)/exp(jt) run ahead of
                        # o/dn(jt-2) so the PE never waits on the ACT exp
                        for step in range(NJT + 2):
                            if step < NJT:
                                jt = step
                                sp = bps.tile([P, IBLK], f32, tag="s")
                                nc.tensor.matmul(
                                    sp[:],
                                    kT_sb[:, h * T + jt * P:h * T + (jt + 1) * P],
                                    q_sl, start=True, stop=True)
                                e = epool.tile([P, IBLK], bf16, tag="e")
                                nc.scalar.activation(e[:], sp[:], Exp, scale=SCALE)
                                es[jt] = e
                            if step >= 2:
                                jt = step - 2
                                e = es.pop(jt)
                                nc.tensor.matmul(
                                    op[:],
                                    v_sb[:, jt * CPC + h * HD:jt * CPC + (h + 1) * HD],
                                    e[:], start=(jt == 0), stop=(jt == NJT - 1))
                                nc.tensor.matmul(
                                    dn[:], ones_sb[:], e[:],
                                    start=(jt == 0), stop=(jt == NJT - 1))
                        rcp = rcpool.tile([P, IBLK], f32, tag="rc")
                        nc.vector.reciprocal_approx_fast(out=rcp[:], in_=dn[:])
                        nc.vector.tensor_tensor(
                            oT_sb[:, h * T + i0:h * T + i0 + IBLK],
                            op[:], rcp[:], mult)
                    # output projection for this 512-token block: both heads'
                    # oT are ready, so C's work hides under the next ib's
                    # attention instead of a batch-end tail.
                    for tl in range(IBLK // P):
                        tt = ib * (IBLK // P) + tl
                        yt = ypool.tile([P, D], bf16, tag="yt")
                        for db in range(D // IBLK):
                            yp = bps.tile([P, IBLK], f32, tag="y")
                            for h in range(HPC):
                                nc.tensor.matmul(
                                    yp[:],
                                    oT_sb[:, h * T + tt * P:h * T + (tt + 1) * P],
                                    wo_sb[:, h * D + db * IBLK:h * D + (db + 1) * IBLK],
                                    start=(h == 0), stop=(h == HPC - 1))
                            nc.vector.tensor_copy(
                                yt[:, db * IBLK:(db + 1) * IBLK], yp[:])
                        nc.sync.dma_start(
                            y_d[g0 + tt * P:g0 + (tt + 1) * P, :], yt[:])


# head-dim permutation: 32-partition quadrant q holds rope pairs
# 16q..16q+15 as [16 even dims ; 16 odd dims], so the rope cross-term is a
# within-quadrant 16<->16 stream_shuffle instead of a cross-partition DMA
_QPERM = np.zeros(HD, dtype=np.int64)
for _q in range(4):
    for _j in range(16):
        _QPERM[32 * _q + _j] = 2 * (16 * _q + _j)
        _QPERM[32 * _q + 16 + _j] = 2 * (16 * _q + _j) + 1


def _prep_inputs(x, rope_cos, rope_sin, Wq, Wk, Wv, Wo):
    x = np.asarray(x, dtype=np.float32)
    xT = np.ascontiguousarray(x.reshape(BT, D).T).astype(npbf)
    cosT = np.asarray(rope_cos, dtype=np.float32).T  # [64, T]
    sinT = np.asarray(rope_sin, dtype=np.float32).T
    csa = np.zeros((P, T), dtype=np.float32)
    csb = np.zeros((P, T), dtype=np.float32)
    for q in range(4):
        csa[32 * q:32 * q + 16] = cosT[16 * q:16 * q + 16]
        csa[32 * q + 16:32 * q + 32] = cosT[16 * q:16 * q + 16]
        csb[32 * q:32 * q + 16] = -sinT[16 * q:16 * q + 16]
        csb[32 * q + 16:32 * q + 32] = sinT[16 * q:16 * q + 16]
    csa = csa.astype(npbf)
    csb = csb.astype(npbf)
    ones = np.ones((P, P), dtype=npbf)
    Wq = np.asarray(Wq, dtype=np.float32)
    Wk = np.asarray(Wk, dtype=np.float32)
    Wv = np.asarray(Wv, dtype=np.float32)
    Wo = np.asarray(Wo, dtype=np.float32)

    in_maps = []
    for c in range(NCORES):
        cols = slice(c * CPC, (c + 1) * CPC)
        wq_c = Wq[:, cols].reshape(D, HPC, HD)[:, :, _QPERM].reshape(D, CPC)
        wk_c = Wk[:, cols].reshape(D, HPC, HD)[:, :, _QPERM].reshape(D, CPC)
        in_maps.append({
            "xT": xT,
            "wq": np.ascontiguousarray(wq_c).astype(npbf),
            "wk": np.ascontiguousarray(wk_c).astype(npbf),
            "wv": np.ascontiguousarray(Wv[:, cols]).astype(npbf),
            "wo": np.ascontiguousarray(Wo[cols, :]).astype(npbf),
            "csa": csa,
            "csb": csb,
            "ones": ones,
        })
    return in_maps


def kernel(x, rope_cos, rope_sin, Wq, Wk, Wv, Wo, _trace=False):
    global last_results
    if "nc" not in _compiled:
        _compiled["nc"] = _build()
    nc = _compiled["nc"]
    in_maps = _prep_inputs(x, rope_cos, rope_sin, Wq, Wk, Wv, Wo)
    res = run_bass_kernel_spmd(
        nc, in_maps, core_ids=list(range(NCORES)), trace=_trace)
    last_results = res
    y = np.sum(
        np.stack([np.asarray(res.results[c]["y"]).astype(np.float32)
                  for c in range(NCORES)]),
        axis=0, dtype=np.float64)
    return y.reshape(B, T, D).astype(np.float32)


# revision 10
# speedup vs baseline: 1.0271x; 1.0271x over previous
"""Tensor-parallel self-attention (full-attention + rope) Bass kernel, TRN2.

Sharding: 16 heads over 8 cores (2 heads/core). Each core computes its heads'
QKV projections, rope, full attention, and its partial output projection
(rows of Wo for its heads); the host sums the 8 partial outputs (the
all-reduce of the tensor-parallel pattern, done at gather time).

v2 (bf16): all matmul operands are bf16 (same PE rate as f32r but enables
fast-weight-load, halves DMA/SBUF), PSUM accumulation fp32.

Per-core layouts (tokens on the free axis):
  xT      [D=2048, B*T=4096] bf16   x transposed (host-prepped), replicated
  wq/wk   [2048, 256] bf16          head-column shard; within each head the
                                    128 cols are permuted so rope pairs sit
                                    16-even+16-odd per 32-partition quadrant
  wv      [2048, 256] bf16          natural column shard
  wo      [256, 2048] bf16          natural row shard
  csa     [128, 2048] bf16          cos rows replicated on both quadrant halves
  csb     [128, 2048] bf16          [-sin ; +sin] per quadrant half
  ones    [128, 128] bf16           all-ones (softmax denominator matmul)

Rope per tile (no cross-partition DMA): with the quadrant pairing,
  out = pps*csa - stream_shuffle(pps*csb, swap 16<->16 within quadrants)
gives both rotated halves in one subtract (sign folded into csb).

Pipeline per batch: A) QKV projections (512-token blocks, x prefetched on the
ACT dma queue); B) per (head, 512-query block): s=kT.T@q -> exp (ACT, bf16
out) -> o += v.T@e, denom += ones.T@e (software-pipelined 2 deep);
oT = o * recip(denom); C) per 128-token tile: y = sum_h oT_h.T@wo_h,
DVE-copied to a [128,2048] bf16 staging tile, one row-contiguous DMA out.
"""

import sys

sys.path.insert(0, "/opt/trn_rl_repo")

import numpy as np
import ml_dtypes

import concourse.bass as bass
import concourse.mybir as mybir
import concourse.tile as tile
from concourse import bacc
from concourse.bass_utils import run_bass_kernel_spmd

B, T, D = 2, 2048, 2048
NH, HD = 16, 128
NCORES = 8
HPC = NH // NCORES          # heads per core = 2
CPC = HPC * HD              # proj columns per core = 256
BT = B * T                  # 4096 tokens
P = 128
TBLK = 512                  # phase-A token block
NBLK = T // TBLK            # 4 blocks per batch
DC = D // P                 # 16 contraction chunks
IBLK = 512                  # phase-B query block
NIB = T // IBLK             # 4 i-blocks per batch
NJT = T // P                # 16 key tiles per batch
SCALE = 1.0 / float(np.sqrt(HD))
SWAP_MASK = list(range(16, 32)) + list(range(16))

f32 = mybir.dt.float32
bf16 = mybir.dt.bfloat16
npbf = ml_dtypes.bfloat16

_compiled = {}

# exposed for test.py
last_results = None


def _build():
    nc = bacc.Bacc("TRN2", target_bir_lowering=False, debug=False)

    xT_d = nc.dram_tensor("xT", [D, BT], bf16, kind="ExternalInput").ap()
    wq_d = nc.dram_tensor("wq", [D, CPC], bf16, kind="ExternalInput").ap()
    wk_d = nc.dram_tensor("wk", [D, CPC], bf16, kind="ExternalInput").ap()
    wv_d = nc.dram_tensor("wv", [D, CPC], bf16, kind="ExternalInput").ap()
    wo_d = nc.dram_tensor("wo", [CPC, D], bf16, kind="ExternalInput").ap()
    csa_d = nc.dram_tensor("csa", [P, T], bf16, kind="ExternalInput").ap()
    csb_d = nc.dram_tensor("csb", [P, T], bf16, kind="ExternalInput").ap()
    ones_d = nc.dram_tensor("ones", [P, P], bf16, kind="ExternalInput").ap()
    y_d = nc.dram_tensor("y", [BT, D], bf16, kind="ExternalOutput").ap()

    with tile.TileContext(nc) as tc:
        _emit(nc, tc, xT_d, wq_d, wk_d, wv_d, wo_d, csa_d, csb_d, ones_d, y_d)
    nc.compile()
    return nc


def _emit(nc, tc, xT_d, wq_d, wk_d, wv_d, wo_d, csa_d, csb_d, ones_d, y_d):
    from contextlib import ExitStack

    Exp = mybir.ActivationFunctionType.Exp
    mult = mybir.AluOpType.mult
    sub = mybir.AluOpType.subtract

    with ExitStack() as ctx:
        const = ctx.enter_context(tc.tile_pool(name="const", bufs=1))
        state = ctx.enter_context(tc.tile_pool(name="state", bufs=1))

        wq_sb = const.tile([P, DC * CPC], bf16, tag="wq")
        wk_sb = const.tile([P, DC * CPC], bf16, tag="wk")
        wv_sb = const.tile([P, DC * CPC], bf16, tag="wv")
        wo_sb = const.tile([P, HPC * D], bf16, tag="wo")
        csa_sb = const.tile([P, T], bf16, tag="csa")
        csb_sb = const.tile([P, T], bf16, tag="csb")
        ones_sb = const.tile([P, P], bf16, tag="ones")

        # weights in consumption-ordered pieces over two dma queues (wq on
        # sync, wk on gpsimd): a tiny first piece unblocks the first matmul
        # in ~2us, bigger pieces amortize the ~0.6us per-DMA issue overhead
        def wpieces(q, dst, src, pieces=(1, 7, 8)):
            dstr = dst[:].rearrange("p (dc c) -> p dc c", dc=DC)
            srcr = src.rearrange("(dc p) c -> p dc c", p=P)
            dc0 = 0
            for n in pieces:
                q.dma_start(dstr[:, dc0:dc0 + n, :], srcr[:, dc0:dc0 + n, :])
                dc0 += n

        wpieces(nc.sync, wq_sb, wq_d)
        wpieces(nc.gpsimd, wk_sb, wk_d)
        nc.gpsimd.dma_start(csa_sb[:], csa_d[:])
        nc.gpsimd.dma_start(csb_sb[:], csb_d[:])
        wpieces(nc.sync, wv_sb, wv_d, pieces=(8, 8))
        nc.sync.dma_start(ones_sb[:], ones_d[:])
        for h in range(HPC):
            nc.gpsimd.dma_start(
                wo_sb[:, h * D:(h + 1) * D], wo_d[h * P:(h + 1) * P, :])

        qT_sb = state.tile([P, HPC * T], bf16, tag="qT")
        kT_sb = state.tile([P, HPC * T], bf16, tag="kT")
        v_sb = state.tile([P, NJT * CPC], bf16, tag="v")
        oT_sb = state.tile([P, HPC * T], bf16, tag="oT")

        xpool = ctx.enter_context(tc.tile_pool(name="xa", bufs=3))
        xT_src = xT_d.rearrange("(dc p) t -> p dc t", p=P)
        ablocks = [(b, blk) for b in range(B) for blk in range(NBLK)]
        xtiles = {}

        def issue_x(i):
            # x DMAs ride the ACT hwdge queue (idle during phase A, and in
            # phase B the next batch's prefetch lands between exp calls);
            # finer pieces for block 0 so the first matmuls aren't DMA-paced
            if i >= len(ablocks):
                return
            bb, blk = ablocks[i]
            xt = xpool.tile([P, DC * TBLK], bf16, tag="x")
            xr = xt[:].rearrange("p (dc t) -> p dc t", dc=DC)
            t0 = bb * T + blk * TBLK
            step = 2 if i == 0 else 4
            for c0 in range(0, DC, step):
                nc.scalar.dma_start(
                    xr[:, c0:c0 + step, :],
                    xT_src[:, c0:c0 + step, t0:t0 + TBLK])
            xtiles[i] = xt

        issue_x(0)
        issue_x(1)

        wqr = wq_sb[:].rearrange("p (dc c) -> p dc c", dc=DC)
        wkr = wk_sb[:].rearrange("p (dc c) -> p dc c", dc=DC)
        wvr = wv_sb[:].rearrange("p (dc c) -> p dc c", dc=DC)

        for b in range(B):
            g0 = b * T

            with tc.tile_pool(name=f"ra{b}", bufs=6) as rpool, \
                 tc.tile_pool(name=f"qk_ps{b}", bufs=3, space="PSUM") as qkps, \
                 tc.tile_pool(name=f"v_ps{b}", bufs=2, space="PSUM") as vps:
                for blk in range(NBLK):
                    idx = b * NBLK + blk
                    issue_x(idx + 2)
                    xt = xtiles.pop(idx)
                    xr = xt[:].rearrange("p (dc t) -> p dc t", dc=DC)
                    t0 = blk * TBLK
                    for h in range(HPC):
                        for wr, dst in ((wqr, qT_sb), (wkr, kT_sb)):
                            pps = qkps.tile([P, TBLK], f32, tag="qk")
                            for dc in range(DC):
                                nc.tensor.matmul(
                                    pps[:],
                                    wr[:, dc, h * HD:(h + 1) * HD],
                                    xr[:, dc, :],
                                    start=(dc == 0), stop=(dc == DC - 1))
                            ra = rpool.tile([P, TBLK], f32, tag="ra")
                            rb = rpool.tile([P, TBLK], f32, tag="rb")
                            rs = rpool.tile([P, TBLK], f32, tag="rs")
                            nc.vector.tensor_tensor(
                                ra[:], pps[:], csa_sb[:, t0:t0 + TBLK], mult)
                            nc.vector.tensor_tensor(
                                rb[:], pps[:], csb_sb[:, t0:t0 + TBLK], mult)
                            nc.vector.stream_shuffle(rs[:], rb[:], SWAP_MASK)
                            nc.vector.tensor_tensor(
                                dst[:, h * T + t0:h * T + t0 + TBLK],
                                ra[:], rs[:], sub)
                    for vi in range(2):
                        vp = vps.tile([P, 2 * CPC], f32, tag="v")
                        for tl2 in range(2):
                            tl = vi * 2 + tl2
                            for dc in range(DC):
                                nc.tensor.matmul(
                                    vp[:, tl2 * CPC:(tl2 + 1) * CPC],
                                    xr[:, dc, tl * P:(tl + 1) * P],
                                    wvr[:, dc, :],
                                    start=(dc == 0), stop=(dc == DC - 1))
                        j0 = blk * 4 + vi * 2
                        nc.vector.tensor_copy(
                            v_sb[:, j0 * CPC:(j0 + 2) * CPC], vp[:])

            with tc.tile_pool(name=f"e{b}", bufs=7) as epool, \
                 tc.tile_pool(name=f"rc{b}", bufs=4) as rcpool, \
                 tc.tile_pool(name=f"yb{b}", bufs=3) as ypool, \
                 tc.tile_pool(name=f"s_ps{b}", bufs=2, space="PSUM") as bps, \
                 tc.tile_pool(name=f"od_ps{b}", bufs=2, space="PSUM") as odn:
                DEPTH = 3
                for ib in range(NIB):
                    i0 = ib * IBLK
                    for h in range(HPC):
                        q_sl = qT_sb[:, h * T + i0:h * T + i0 + IBLK]
                        op = odn.tile([P, IBLK], f32, tag="o")
                        dn = odn.tile([P, IBLK], f32, tag="d", bufs=1)
                        es = {}
                        # software-pipeline: s(jt)/exp(jt) run DEPTH ahead of
                        # o/dn(jt-DEPTH) so the PE never waits on the ACT exp
                        for step in range(NJT + DEPTH):
                            if step < NJT:
                                jt = step
                                sp = bps.tile([P, IBLK], f32, tag="s", bufs=3)
                                nc.tensor.matmul(
                                    sp[:],
                                    kT_sb[:, h * T + jt * P:h * T + (jt + 1) * P],
                                    q_sl, start=True, stop=True)
                                e = epool.tile([P, IBLK], bf16, tag="e")
                                nc.scalar.activation(e[:], sp[:], Exp, scale=SCALE)
                                es[jt] = e
                            if step >= DEPTH:
                                jt = step - DEPTH
                                e = es.pop(jt)
                                nc.tensor.matmul(
                                    op[:],
                                    v_sb[:, jt * CPC + h * HD:jt * CPC + (h + 1) * HD],
                                    e[:], start=(jt == 0), stop=(jt == NJT - 1))
                                nc.tensor.matmul(
                                    dn[:], ones_sb[:], e[:],
                                    start=(jt == 0), stop=(jt == NJT - 1))
                        rcp = rcpool.tile([P, IBLK], f32, tag="rc")
                        nc.vector.reciprocal_approx_fast(out=rcp[:], in_=dn[:])
                        nc.vector.tensor_tensor(
                            oT_sb[:, h * T + i0:h * T + i0 + IBLK],
                            op[:], rcp[:], mult)
                    # output projection for this 512-token block: both heads'
                    # oT are ready, so C's work hides under the next ib's
                    # attention instead of a batch-end tail.
                    YB = 512
                    for tl in range(IBLK // P):
                        tt = ib * (IBLK // P) + tl
                        yt = ypool.tile([P, D], bf16, tag="yt")
                        for db in range(D // YB):
                            yp = bps.tile([P, YB], f32, tag="y")
                            for h in range(HPC):
                                nc.tensor.matmul(
                                    yp[:],
                                    oT_sb[:, h * T + tt * P:h * T + (tt + 1) * P],
                                    wo_sb[:, h * D + db * YB:h * D + (db + 1) * YB],
                                    start=(h == 0), stop=(h == HPC - 1))
                            nc.vector.tensor_copy(
                                yt[:, db * YB:(db + 1) * YB], yp[:])
                        nc.sync.dma_start(
                            y_d[g0 + tt * P:g0 + (tt + 1) * P, :], yt[:])


# head-dim permutation: 32-partition quadrant q holds rope pairs
# 16q..16q+15 as [16 even dims ; 16 odd dims], so the rope cross-term is a
# within-quadrant 16<->16 stream_shuffle instead of a cross-partition DMA
_QPERM = np.zeros(HD, dtype=np.int64)
for _q in range(4):
    for _j in range(16):
        _QPERM[32 * _q + _j] = 2 * (16 * _q + _j)
        _QPERM[32 * _q + 16 + _j] = 2 * (16 * _q + _j) + 1


def _prep_inputs(x, rope_cos, rope_sin, Wq, Wk, Wv, Wo):
    x = np.asarray(x, dtype=np.float32)
    xT = np.ascontiguousarray(x.reshape(BT, D).T).astype(npbf)
    cosT = np.asarray(rope_cos, dtype=np.float32).T  # [64, T]
    sinT = np.asarray(rope_sin, dtype=np.float32).T
    csa = np.zeros((P, T), dtype=np.float32)
    csb = np.zeros((P, T), dtype=np.float32)
    for q in range(4):
        csa[32 * q:32 * q + 16] = cosT[16 * q:16 * q + 16]
        csa[32 * q + 16:32 * q + 32] = cosT[16 * q:16 * q + 16]
        csb[32 * q:32 * q + 16] = -sinT[16 * q:16 * q + 16]
        csb[32 * q + 16:32 * q + 32] = sinT[16 * q:16 * q + 16]
    csa = csa.astype(npbf)
    csb = csb.astype(npbf)
    ones = np.ones((P, P), dtype=npbf)
    Wq = np.asarray(Wq, dtype=np.float32)
    Wk = np.asarray(Wk, dtype=np.float32)
    Wv = np.asarray(Wv, dtype=np.float32)
    Wo = np.asarray(Wo, dtype=np.float32)

    in_maps = []
    for c in range(NCORES):
        cols = slice(c * CPC, (c + 1) * CPC)
        wq_c = Wq[:, cols].reshape(D, HPC, HD)[:, :, _QPERM].reshape(D, CPC)
        wk_c = Wk[:, cols].reshape(D, HPC, HD)[:, :, _QPERM].reshape(D, CPC)
        in_maps.append({
            "xT": xT,
            "wq": np.ascontiguousarray(wq_c).astype(npbf),
            "wk": np.ascontiguousarray(wk_c).astype(npbf),
            "wv": np.ascontiguousarray(Wv[:, cols]).astype(npbf),
            "wo": np.ascontiguousarray(Wo[cols, :]).astype(npbf),
            "csa": csa,
            "csb": csb,
            "ones": ones,
        })
    return in_maps


def kernel(x, rope_cos, rope_sin, Wq, Wk, Wv, Wo, _trace=False):
    global last_results
    if "nc" not in _compiled:
        _compiled["nc"] = _build()
    nc = _compiled["nc"]
    in_maps = _prep_inputs(x, rope_cos, rope_sin, Wq, Wk, Wv, Wo)
    res = run_bass_kernel_spmd(
        nc, in_maps, core_ids=list(range(NCORES)), trace=_trace)
    last_results = res
    y = np.sum(
        np.stack([np.asarray(res.results[c]["y"]).astype(np.float32)
                  for c in range(NCORES)]),
        axis=0, dtype=np.float64)
    return y.reshape(B, T, D).astype(np.float32)
